# revision 31
# baseline (speedup 1.0000x reference)
"""Trainium2 Bass kernel for nn_NeuralNet_19516331393457 (dense_mlp).

Pipeline: x = embed[data] (48-entry table); h1 = relu(x@W1+b1);
h2 = tanh(h1@W2+b2); out = h2@W3+b3; return out[argmax(F(out0, out1))].

v3 strategy (data-parallel over N=500000 on 8 cores, bf16 device math):
  - Host: tiny-table gather embed[data] in bf16, tile-blocked transpose to
    [NQ, 128, 2048] per core (quads of 4 chunks x 512 samples).
  - Device, software-pipelined per quad q: MM1(q) x4 -> 2x [128,1024] PSUM;
    relu(q) split DVE/ACT -> h1 bf16; then the *previous* quad's tail:
    MM2(q-1) x4 column-packed 2-up into one [128,1024] PSUM; one ACT tanh
    evicts 4 chunks; MM3 via block-diag W3blk=[W3 0; 0 W3] -> dense [4,512]
    outputs at partition 32p of a per-2-quad po bank; one DVE cast per 2
    quads -> fp16 staging; strided output DMAs per 8-quad piece.
  - Host: decode outs, F in fp64, exact top-K rescore in fp32 (device
    ordering only needs the winner inside the top-K shortlist).
"""

import numpy as np
import ml_dtypes

import concourse.mybir as mybir
import concourse.tile as tile
from concourse import bacc
from concourse.bass_utils import run_bass_kernel_spmd

N = 500000
D = 128
H1 = 128
H2 = 64
NCLS = 2
NCORES = 8
CHUNK = 512
NPC_RAW = N // NCORES              # 62500 samples per core
NQ = 31                            # quads per core (4 chunks each)
CHUNKS = 4 * NQ                    # 124
NPC = CHUNKS * CHUNK               # 63488 padded samples per core
NG = 16                            # 2-quad groups (last is half)
PIECE = 8                          # quads per output staging piece
NPIECE = 4

_F32 = mybir.dt.float32
_F16 = mybir.dt.float16
_BF16 = mybir.dt.bfloat16


def _issue_x_dma(nc, q, pools, tls, xts):
    (xpool, h1pool, h2pool, obs_pool, p1pool, p2pool, popool) = pools
    xt = xpool.tile([D, 4 * CHUNK], _BF16, name=f"xt{q}", tag="xt")
    nc.sync.dma_start(xt[:], tls["x_t"][q, :, :])
    xts[q] = [xt[:, c * CHUNK : (c + 1) * CHUNK] for c in range(4)]


def _quad_head(nc, q, pools, tls, xts):
    """Issue MM1 + relu for quad q (x DMA pre-issued); prefetch x of q+2."""
    (xpool, h1pool, h2pool, obs_pool, p1pool, p2pool, popool) = pools
    xcs = xts.pop(q)

    p1a = p1pool.tile([H1, 2 * CHUNK], _F32, name=f"p1a{q}", tag="p1")
    p1b = p1pool.tile([H1, 2 * CHUNK], _F32, name=f"p1b{q}", tag="p1")
    for h, p1t in ((0, p1a), (1, p1b)):
        for s in range(2):
            nc.tensor.matmul(
                p1t[:, s * CHUNK : (s + 1) * CHUNK],
                tls["w1sb"],
                xcs[2 * h + s],
                start=True, stop=True,
            )

    h1a = h1pool.tile([H1, 2 * CHUNK], _BF16, name=f"h1a{q}", tag="h1")
    nc.vector.tensor_scalar(
        h1a[:], p1a[:], tls["b1sb"], 0.0,
        mybir.AluOpType.add, mybir.AluOpType.max,
    )
    h1b = h1pool.tile([H1, 2 * CHUNK], _BF16, name=f"h1b{q}", tag="h1")
    if q % 4 == 3:
        nc.vector.tensor_scalar(
            h1b[:], p1b[:], tls["b1sb"], 0.0,
            mybir.AluOpType.add, mybir.AluOpType.max,
        )
    else:
        nc.scalar.activation(
            h1b[:], p1b[:], mybir.ActivationFunctionType.Relu,
            bias=tls["b1sb"],
        )
    return h1a, h1b


def _quad_tail(nc, q, pools, tls, h1ab, pos, obs):
    """Issue MM2 + tanh + MM3 + (cast + out-DMA) for quad q."""
    (xpool, h1pool, h2pool, obpool, p1pool, p2pool, popool) = pools
    h1a, h1b = h1ab
    p2 = p2pool.tile([128, 2 * CHUNK], _F32, name=f"p2_{q}", tag="p2")
    for h, h1t in ((0, h1a), (1, h1b)):
        for s in range(2):
            nc.tensor.matmul(
                p2[s * H2 : (s + 1) * H2, h * CHUNK : (h + 1) * CHUNK],
                tls["w2sb"],
                h1t[:, s * CHUNK : (s + 1) * CHUNK],
                start=True, stop=True,
            )

    h2t = h2pool.tile([128, 2 * CHUNK], _BF16, name=f"h2_{q}", tag="h2")
    nc.scalar.activation(
        h2t[:], p2[:], mybir.ActivationFunctionType.Tanh,
        bias=tls["b2sb"],
    )

    # MM3 via block-diag W3blk [128,4]: one matmul per h2 column-pair makes
    # a dense [4,512] output; 4 pairs (2 quads) pack one [*,512] PSUM bank.
    g = q // 2
    if q % 2 == 0:
        pos[g] = popool.tile([128, CHUNK], _F32, name=f"po{g}", tag="po")
    po = pos[g]
    for h in range(2):
        p = 2 * (q % 2) + h
        nc.tensor.matmul(
            po[32 * p : 32 * p + 4, :],
            tls["w3sb"],
            h2t[:, h * CHUNK : (h + 1) * CHUNK],
            start=True, stop=True,
            tile_position=(0, 32 * p),
        )

    if q % 2 == 1 or q == NQ - 1:
        piece = g // 4
        if piece not in obs:
            obs[piece] = obpool.tile(
                [128, 4 * CHUNK], _F16, name=f"ob{piece}", tag="ob",
            )
        slot = g % 4
        nc.vector.tensor_copy(
            obs[piece][0:100, slot * CHUNK : (slot + 1) * CHUNK],
            po[0:100, :],
        )
        if g % 4 == 3 or g == NG - 1:
            cols = (slot + 1) * CHUNK
            base = piece * 4 * CHUNK
            nc.sync.dma_start(
                tls["out_d"][:, base : base + cols],
                obs[piece][0:100, 0:cols],
            )


def _build_bass():
    nc = bacc.Bacc(
        "TRN2",
        target_bir_lowering=False,
        debug=False,
        enable_asserts=False,
        num_devices=NCORES,
    )
    x_t = nc.dram_tensor("x_t", [NQ, D, 4 * CHUNK], _BF16, kind="ExternalInput")
    # packed weights: cols [0:128]=W1, [128:192]=W2, [192:196]=blockdiag W3
    wpk = nc.dram_tensor("wpk", [128, H1 + H2 + 4], _BF16,
                         kind="ExternalInput")
    # packed biases: col 0 = b1, col 1 = [b2; b2]
    bpk = nc.dram_tensor("bpk", [128, 2], _F32, kind="ExternalInput")
    # row 32p+r = class r%2 of in-group chunk 4(p//2)+2(p%2)+r//2, group g
    # at cols [g*512:(g+1)*512] (dense partition dump of the po layout)
    out_d = nc.dram_tensor("out_d", [100, NG * CHUNK], _F16,
                           kind="ExternalOutput")

    with tile.TileContext(nc) as tc:
        with (
            tc.tile_pool(name="w", bufs=1) as wpool,
            tc.tile_pool(name="x", bufs=4) as xpool,
            tc.tile_pool(name="x0", bufs=4) as x0pool,
            tc.tile_pool(name="h1", bufs=4) as h1pool,
            tc.tile_pool(name="h2", bufs=3) as h2pool,
            tc.tile_pool(name="ob", bufs=2) as obpool,
            tc.tile_pool(name="p1", bufs=2, space="PSUM") as p1pool,
            tc.tile_pool(name="p2", bufs=1, space="PSUM") as p2pool,
            tc.tile_pool(name="po", bufs=2, space="PSUM") as popool,
        ):
            tls = {"x_t": x_t, "out_d": out_d}
            pools = (xpool, h1pool, h2pool, obpool, p1pool, p2pool, popool)
            xts = {}

            # quad 0's x first as four quarter-tiles (first MM1 waits 128KB)
            x0q = []
            for h in range(4):
                t = x0pool.tile([D, CHUNK], _BF16, name=f"x0_{h}", tag="x0")
                nc.sync.dma_start(
                    t[:], x_t[0, :, h * CHUNK : (h + 1) * CHUNK])
                x0q.append(t)
            xts[0] = x0q
            _issue_x_dma(nc, 1, pools, tls, xts)
            # weights via the GPSIMD (SWDGE) queue, off the x-critical path
            wsb = wpool.tile([128, H1 + H2 + 4], _BF16)
            nc.gpsimd.dma_start(wsb[:], wpk[:, :])
            bsb = wpool.tile([128, 2], _F32)
            nc.gpsimd.dma_start(bsb[:], bpk[:, :])
            tls.update({
                "w1sb": wsb[:, 0:H1], "w2sb": wsb[:, H1 : H1 + H2],
                "w3sb": wsb[:, H1 + H2 : H1 + H2 + 4],
                "b1sb": bsb[:, 0:1], "b2sb": bsb[:, 1:2],
            })

            # tail-first issue order: tanh(q-1)/MM2(q-1) precede relu-b(q)/
            # MM1(q) on their engine queues, matching input-readiness order
            pos, obs = {}, {}
            prev = None
            for q in range(NQ):
                if q + 2 < NQ:
                    _issue_x_dma(nc, q + 2, pools, tls, xts)
                if prev is not None:
                    _quad_tail(nc, q - 1, pools, tls, prev, pos, obs)
                prev = _quad_head(nc, q, pools, tls, xts)
            _quad_tail(nc, NQ - 1, pools, tls, prev, pos, obs)

    nc.compile()
    return nc


_NC_CACHE = None


def _get_nc():
    global _NC_CACHE
    if _NC_CACHE is None:
        _NC_CACHE = _build_bass()
    return _NC_CACHE


def _F64(x, y):
    return (
        3.0 * (1.0 - x) ** 2 * np.exp(-(x**2) - (y + 1.0) ** 2)
        - 10.0 * (x / 5.0 - x**3 - y**5) * np.exp(-(x**2) - y**2)
        - 1.0 / (3.0 ** np.exp(-((x + 1.0) ** 2) - y**2))
    )


def make_in_maps(data, embed, W1, b1, W2, b2, W3, b3):
    data = np.asarray(data)
    table16 = np.asarray(embed, dtype=np.float32).reshape(-1).astype(
        ml_dtypes.bfloat16)
    wpk = np.zeros((128, H1 + H2 + 4), np.float32)
    wpk[:, 0:H1] = np.asarray(W1, np.float32)
    wpk[0:64, H1 : H1 + H2] = np.asarray(W2, np.float32)[0:64]
    wpk[64:128, H1 : H1 + H2] = np.asarray(W2, np.float32)[64:128]
    W3c = np.asarray(W3, np.float32)
    wpk[0:64, H1 + H2 : H1 + H2 + 2] = W3c
    wpk[64:128, H1 + H2 + 2 : H1 + H2 + 4] = W3c
    wpk = np.ascontiguousarray(wpk.astype(ml_dtypes.bfloat16))
    b2c = np.asarray(b2, dtype=np.float32).reshape(H2, 1)
    bpk = np.zeros((128, 2), np.float32)
    bpk[:, 0:1] = np.ascontiguousarray(b1, dtype=np.float32).reshape(H1, 1)
    bpk[:, 1:2] = np.concatenate([b2c, b2c], axis=0)

    in_maps = []
    for c in range(NCORES):
        dshard = data[c * NPC_RAW : (c + 1) * NPC_RAW]
        dpad = np.zeros((NPC, D), dtype=dshard.dtype)
        dpad[:NPC_RAW] = dshard
        xt = np.ascontiguousarray(
            table16[dpad.reshape(NQ, 4 * CHUNK, D).transpose(0, 2, 1)]
        )
        in_maps.append({"x_t": xt, "wpk": wpk, "bpk": bpk})
    return in_maps


def _decode_outs(res):
    """-> out0_all, out1_all fp32 arrays of shape [N] (padding stripped)."""
    o0s, o1s = [], []
    for c in range(NCORES):
        od = np.asarray(res.results[c]["out_d"], np.float32)
        arr = od.reshape(100, NG, CHUNK)            # [row, g, i]
        o0 = np.empty((CHUNKS, CHUNK), np.float32)
        o1 = np.empty((CHUNKS, CHUNK), np.float32)
        for r in range(4):
            for p in range(4):
                ch = 4 * (p // 2) + 2 * (p % 2) + (r // 2)  # chunk-in-group
                dst = o0 if r % 2 == 0 else o1
                ks = np.arange(NG) * 8 + ch
                valid = ks < CHUNKS
                dst[ks[valid]] = arr[32 * p + r, valid]
        o0s.append(o0.reshape(-1)[:NPC_RAW])
        o1s.append(o1.reshape(-1)[:NPC_RAW])
    return np.concatenate(o0s), np.concatenate(o1s)


def kernel(data, embed, W1, b1, W2, b2, W3, b3):
    data = np.asarray(data)
    nc = _get_nc()
    in_maps = make_in_maps(data, embed, W1, b1, W2, b2, W3, b3)
    res = run_bass_kernel_spmd(nc, in_maps, core_ids=list(range(NCORES)))
    o0, o1 = _decode_outs(res)

    pred = _F64(o0.astype(np.float64), o1.astype(np.float64))
    K = 4096
    cand = np.argpartition(pred, N - K)[N - K:]

    table32 = np.asarray(embed, dtype=np.float32).reshape(-1)
    W1f = np.asarray(W1, np.float32)
    W2f = np.asarray(W2, np.float32)
    W3f = np.asarray(W3, np.float32)
    xk = table32[data[cand]]
    hk = np.maximum(xk @ W1f + np.asarray(b1, np.float32), 0.0)
    hk = np.tanh(hk @ W2f + np.asarray(b2, np.float32))
    ok = hk @ W3f + np.asarray(b3, np.float32)
    pk = _F64(ok[:, 0].astype(np.float64), ok[:, 1].astype(np.float64))
    return ok[int(np.argmax(pk))].astype(np.float32)


# revision 33
# speedup vs baseline: 1.0767x; 1.0767x over previous
"""Trainium2 Bass kernel for nn_NeuralNet_19516331393457 (dense_mlp).

Pipeline: x = embed[data] (48-entry table); h1 = relu(x@W1+b1);
h2 = tanh(h1@W2+b2); out = h2@W3+b3; return out[argmax(F(out0, out1))].

v3 strategy (data-parallel over N=500000 on 8 cores, bf16 device math):
  - Host: tiny-table gather embed[data] in bf16, tile-blocked transpose to
    [NQ, 128, 2048] per core (quads of 4 chunks x 512 samples).
  - Device, software-pipelined per quad q: MM1(q) x4 -> 2x [128,1024] PSUM;
    relu(q) split DVE/ACT -> h1 bf16; then the *previous* quad's tail:
    MM2(q-1) x4 column-packed 2-up into one [128,1024] PSUM; one ACT tanh
    evicts 4 chunks; MM3 via block-diag W3blk=[W3 0; 0 W3] -> dense [4,512]
    outputs at partition 32p of a per-2-quad po bank; one DVE cast per 2
    quads -> fp16 staging; strided output DMAs per 8-quad piece.
  - Host: decode outs, F in fp64, exact top-K rescore in fp32 (device
    ordering only needs the winner inside the top-K shortlist).
"""

import numpy as np
import ml_dtypes

import concourse.mybir as mybir
import concourse.tile as tile
from concourse import bacc
from concourse.bass_utils import run_bass_kernel_spmd

N = 500000
D = 128
H1 = 128
H2 = 64
NCLS = 2
NCORES = 8
CHUNK = 512
NPC_RAW = N // NCORES              # 62500 samples per core
NQ = 31                            # quads per core (4 chunks each)
CHUNKS = 4 * NQ                    # 124
NPC = CHUNKS * CHUNK               # 63488 padded samples per core
NG = 16                            # 2-quad groups (last is half)
PIECE = 8                          # quads per output staging piece
NPIECE = 4

_F32 = mybir.dt.float32
_F16 = mybir.dt.float16
_BF16 = mybir.dt.bfloat16


def _issue_x_dma(nc, q, pools, tls, xts):
    (xpool, h1pool, h2pool, obs_pool, p1pool, p2pool, popool) = pools
    xt = xpool.tile([D, 4 * CHUNK], _BF16, name=f"xt{q}", tag="xt")
    nc.sync.dma_start(xt[:], tls["x_t"][q, :, :])
    xts[q] = [xt[:, c * CHUNK : (c + 1) * CHUNK] for c in range(4)]


def _quad_head_mm(nc, q, pools, tls, xts):
    """Issue MM1 for quad q (x DMA pre-issued)."""
    (xpool, h1pool, h2pool, obs_pool, p1pool, p2pool, popool) = pools
    xcs = xts.pop(q)

    p1a = p1pool.tile([H1, 2 * CHUNK], _F32, name=f"p1a{q}", tag="p1")
    p1b = p1pool.tile([H1, 2 * CHUNK], _F32, name=f"p1b{q}", tag="p1")
    for h, p1t in ((0, p1a), (1, p1b)):
        for s in range(2):
            nc.tensor.matmul(
                p1t[:, s * CHUNK : (s + 1) * CHUNK],
                tls["w1sb"],
                xcs[2 * h + s],
                start=True, stop=True,
            )
    return p1a, p1b


def _quad_head_relu(nc, q, pools, tls, p1ab):
    (xpool, h1pool, h2pool, obs_pool, p1pool, p2pool, popool) = pools
    p1a, p1b = p1ab
    h1a = h1pool.tile([H1, 2 * CHUNK], _BF16, name=f"h1a{q}", tag="h1")
    nc.vector.tensor_scalar(
        h1a[:], p1a[:], tls["b1sb"], 0.0,
        mybir.AluOpType.add, mybir.AluOpType.max,
    )
    h1b = h1pool.tile([H1, 2 * CHUNK], _BF16, name=f"h1b{q}", tag="h1")
    if q % 4 == 3:
        nc.vector.tensor_scalar(
            h1b[:], p1b[:], tls["b1sb"], 0.0,
            mybir.AluOpType.add, mybir.AluOpType.max,
        )
    else:
        nc.scalar.activation(
            h1b[:], p1b[:], mybir.ActivationFunctionType.Relu,
            bias=tls["b1sb"],
        )
    return h1a, h1b


def _quad_tail_a(nc, q, pools, tls, h1ab):
    """Issue MM2 + tanh for quad q."""
    (xpool, h1pool, h2pool, obpool, p1pool, p2pool, popool) = pools
    h1a, h1b = h1ab
    p2 = p2pool.tile([128, 2 * CHUNK], _F32, name=f"p2_{q}", tag="p2")
    for h, h1t in ((0, h1a), (1, h1b)):
        for s in range(2):
            nc.tensor.matmul(
                p2[s * H2 : (s + 1) * H2, h * CHUNK : (h + 1) * CHUNK],
                tls["w2sb"],
                h1t[:, s * CHUNK : (s + 1) * CHUNK],
                start=True, stop=True,
            )

    h2t = h2pool.tile([128, 2 * CHUNK], _BF16, name=f"h2_{q}", tag="h2")
    nc.scalar.activation(
        h2t[:], p2[:], mybir.ActivationFunctionType.Tanh,
        bias=tls["b2sb"],
    )
    return h2t


def _quad_tail_b(nc, q, pools, tls, h2t, pos, obs):
    """Issue MM3 + (cast + out-DMA) for quad q."""
    (xpool, h1pool, h2pool, obpool, p1pool, p2pool, popool) = pools
    # MM3 via block-diag W3blk [128,4]: one matmul per h2 column-pair makes
    # a dense [4,512] output; 4 pairs (2 quads) pack one [*,512] PSUM bank.
    g = q // 2
    if q % 2 == 0:
        pos[g] = popool.tile([128, CHUNK], _F32, name=f"po{g}", tag="po")
    po = pos[g]
    for h in range(2):
        p = 2 * (q % 2) + h
        nc.tensor.matmul(
            po[32 * p : 32 * p + 4, :],
            tls["w3sb"],
            h2t[:, h * CHUNK : (h + 1) * CHUNK],
            start=True, stop=True,
            tile_position=(0, 32 * p),
        )

    if q % 2 == 1 or q == NQ - 1:
        piece = g // 4
        if piece not in obs:
            obs[piece] = obpool.tile(
                [128, 4 * CHUNK], _F16, name=f"ob{piece}", tag="ob",
            )
        slot = g % 4
        nc.vector.tensor_copy(
            obs[piece][0:100, slot * CHUNK : (slot + 1) * CHUNK],
            po[0:100, :],
        )
        if g % 4 == 3 or g == NG - 1:
            cols = (slot + 1) * CHUNK
            base = piece * 4 * CHUNK
            nc.sync.dma_start(
                tls["out_d"][:, base : base + cols],
                obs[piece][0:100, 0:cols],
            )


def _build_bass():
    nc = bacc.Bacc(
        "TRN2",
        target_bir_lowering=False,
        debug=False,
        enable_asserts=False,
        num_devices=NCORES,
    )
    x_t = nc.dram_tensor("x_t", [NQ, D, 4 * CHUNK], _BF16, kind="ExternalInput")
    # packed weights: cols [0:128]=W1, [128:192]=W2, [192:196]=blockdiag W3
    wpk = nc.dram_tensor("wpk", [128, H1 + H2 + 4], _BF16,
                         kind="ExternalInput")
    # packed biases: col 0 = b1, col 1 = [b2; b2]
    bpk = nc.dram_tensor("bpk", [128, 2], _F32, kind="ExternalInput")
    # row 32p+r = class r%2 of in-group chunk 4(p//2)+2(p%2)+r//2, group g
    # at cols [g*512:(g+1)*512] (dense partition dump of the po layout)
    out_d = nc.dram_tensor("out_d", [100, NG * CHUNK], _F16,
                           kind="ExternalOutput")

    with tile.TileContext(nc) as tc:
        with (
            tc.tile_pool(name="w", bufs=1) as wpool,
            tc.tile_pool(name="x", bufs=4) as xpool,
            tc.tile_pool(name="x0", bufs=4) as x0pool,
            tc.tile_pool(name="h1", bufs=4) as h1pool,
            tc.tile_pool(name="h2", bufs=3) as h2pool,
            tc.tile_pool(name="ob", bufs=2) as obpool,
            tc.tile_pool(name="p1", bufs=2, space="PSUM") as p1pool,
            tc.tile_pool(name="p2", bufs=1, space="PSUM") as p2pool,
            tc.tile_pool(name="po", bufs=2, space="PSUM") as popool,
        ):
            tls = {"x_t": x_t, "out_d": out_d}
            pools = (xpool, h1pool, h2pool, obpool, p1pool, p2pool, popool)
            xts = {}

            # quad 0's x first as four quarter-tiles (first MM1 waits 128KB)
            x0q = []
            for h in range(4):
                t = x0pool.tile([D, CHUNK], _BF16, name=f"x0_{h}", tag="x0")
                nc.sync.dma_start(
                    t[:], x_t[0, :, h * CHUNK : (h + 1) * CHUNK])
                x0q.append(t)
            xts[0] = x0q
            _issue_x_dma(nc, 1, pools, tls, xts)
            # weights via the GPSIMD (SWDGE) queue, off the x-critical path
            wsb = wpool.tile([128, H1 + H2 + 4], _BF16)
            nc.gpsimd.dma_start(wsb[:], wpk[:, :])
            bsb = wpool.tile([128, 2], _F32)
            nc.gpsimd.dma_start(bsb[:], bpk[:, :])
            tls.update({
                "w1sb": wsb[:, 0:H1], "w2sb": wsb[:, H1 : H1 + H2],
                "w3sb": wsb[:, H1 + H2 : H1 + H2 + 4],
                "b1sb": bsb[:, 0:1], "b2sb": bsb[:, 1:2],
            })

            # interleaved issue so each engine queue matches input-readiness:
            # PE: MM1(q), MM2(q-1), MM3(q-1); ACT: tanh(q-1), relu-b(q);
            # DVE: relu-a(q), cast(q-1)
            pos, obs = {}, {}
            prev_h1 = None
            prev_h2 = None
            for q in range(NQ):
                if q + 2 < NQ:
                    _issue_x_dma(nc, q + 2, pools, tls, xts)
                p1ab = _quad_head_mm(nc, q, pools, tls, xts)
                if prev_h1 is not None:
                    prev_h2 = (q - 1, _quad_tail_a(nc, q - 1, pools, tls,
                                                   prev_h1))
                prev_h1 = _quad_head_relu(nc, q, pools, tls, p1ab)
                if prev_h2 is not None:
                    _quad_tail_b(nc, prev_h2[0], pools, tls, prev_h2[1],
                                 pos, obs)
                    prev_h2 = None
            h2last = _quad_tail_a(nc, NQ - 1, pools, tls, prev_h1)
            _quad_tail_b(nc, NQ - 1, pools, tls, h2last, pos, obs)

    nc.compile()
    return nc


_NC_CACHE = None


def _get_nc():
    global _NC_CACHE
    if _NC_CACHE is None:
        _NC_CACHE = _build_bass()
    return _NC_CACHE


def _F64(x, y):
    return (
        3.0 * (1.0 - x) ** 2 * np.exp(-(x**2) - (y + 1.0) ** 2)
        - 10.0 * (x / 5.0 - x**3 - y**5) * np.exp(-(x**2) - y**2)
        - 1.0 / (3.0 ** np.exp(-((x + 1.0) ** 2) - y**2))
    )


def make_in_maps(data, embed, W1, b1, W2, b2, W3, b3):
    data = np.asarray(data)
    table16 = np.asarray(embed, dtype=np.float32).reshape(-1).astype(
        ml_dtypes.bfloat16)
    wpk = np.zeros((128, H1 + H2 + 4), np.float32)
    wpk[:, 0:H1] = np.asarray(W1, np.float32)
    wpk[0:64, H1 : H1 + H2] = np.asarray(W2, np.float32)[0:64]
    wpk[64:128, H1 : H1 + H2] = np.asarray(W2, np.float32)[64:128]
    W3c = np.asarray(W3, np.float32)
    wpk[0:64, H1 + H2 : H1 + H2 + 2] = W3c
    wpk[64:128, H1 + H2 + 2 : H1 + H2 + 4] = W3c
    wpk = np.ascontiguousarray(wpk.astype(ml_dtypes.bfloat16))
    b2c = np.asarray(b2, dtype=np.float32).reshape(H2, 1)
    bpk = np.zeros((128, 2), np.float32)
    bpk[:, 0:1] = np.ascontiguousarray(b1, dtype=np.float32).reshape(H1, 1)
    bpk[:, 1:2] = np.concatenate([b2c, b2c], axis=0)

    in_maps = []
    for c in range(NCORES):
        dshard = data[c * NPC_RAW : (c + 1) * NPC_RAW]
        dpad = np.zeros((NPC, D), dtype=dshard.dtype)
        dpad[:NPC_RAW] = dshard
        xt = np.ascontiguousarray(
            table16[dpad.reshape(NQ, 4 * CHUNK, D).transpose(0, 2, 1)]
        )
        in_maps.append({"x_t": xt, "wpk": wpk, "bpk": bpk})
    return in_maps


def _decode_outs(res):
    """-> out0_all, out1_all fp32 arrays of shape [N] (padding stripped)."""
    o0s, o1s = [], []
    for c in range(NCORES):
        od = np.asarray(res.results[c]["out_d"], np.float32)
        arr = od.reshape(100, NG, CHUNK)            # [row, g, i]
        o0 = np.empty((CHUNKS, CHUNK), np.float32)
        o1 = np.empty((CHUNKS, CHUNK), np.float32)
        for r in range(4):
            for p in range(4):
                ch = 4 * (p // 2) + 2 * (p % 2) + (r // 2)  # chunk-in-group
                dst = o0 if r % 2 == 0 else o1
                ks = np.arange(NG) * 8 + ch
                valid = ks < CHUNKS
                dst[ks[valid]] = arr[32 * p + r, valid]
        o0s.append(o0.reshape(-1)[:NPC_RAW])
        o1s.append(o1.reshape(-1)[:NPC_RAW])
    return np.concatenate(o0s), np.concatenate(o1s)


def kernel(data, embed, W1, b1, W2, b2, W3, b3):
    data = np.asarray(data)
    nc = _get_nc()
    in_maps = make_in_maps(data, embed, W1, b1, W2, b2, W3, b3)
    res = run_bass_kernel_spmd(nc, in_maps, core_ids=list(range(NCORES)))
    o0, o1 = _decode_outs(res)

    pred = _F64(o0.astype(np.float64), o1.astype(np.float64))
    K = 4096
    cand = np.argpartition(pred, N - K)[N - K:]

    table32 = np.asarray(embed, dtype=np.float32).reshape(-1)
    W1f = np.asarray(W1, np.float32)
    W2f = np.asarray(W2, np.float32)
    W3f = np.asarray(W3, np.float32)
    xk = table32[data[cand]]
    hk = np.maximum(xk @ W1f + np.asarray(b1, np.float32), 0.0)
    hk = np.tanh(hk @ W2f + np.asarray(b2, np.float32))
    ok = hk @ W3f + np.asarray(b3, np.float32)
    pk = _F64(ok[:, 0].astype(np.float64), ok[:, 1].astype(np.float64))
    return ok[int(np.argmax(pk))].astype(np.float32)
